# revision 50
# baseline (speedup 1.0000x reference)
"""Multi-head QKV block attention for Trainium2, SPMD over 8 NeuronCores.

Problem: X[4,2048,1024], residual[4,2048,1024], wq/wk/wv[1024,1024],
H=16 heads, D=64, softmax scale sqrt(S/H)=sqrt(128).
out = softmax((X wq)(X wk)^T / sqrt(128)) (X wv) + residual, returned twice.

Sharding: core c handles batch b=c//2 and head group g=c%2 (8 heads = 512
feature columns). Fully data/tensor-parallel -- no collectives; host
assembles the output. X is pre-transposed and bf16-cast on the host
(input marshaling) so the device never spends PE time transposing it.

Per-core kernel (Tile framework), fully fused single phase:
  - K/V/Q projections run on the PE with weights/xT as stationary
    operands, interleaved into the attention stream (K-proj for head pair
    m and V-proj for key block b are injected into earlier iterations'
    t-loops so the PE never drains and ScalarE/VectorE are never idle).
  - per head pair (hp) and 512-query tile (st): transposed logits
    K_h^T.T @ Q_h^T as two K=64 matmuls in disjoint PE row groups; exp is
    split across engines per t-chunk: ScalarE spline exp for 7 chunks,
    VectorE Schraudolph bit-trick exp (tensor_scalar mult+add to int16,
    reinterpreted as bf16 -- exp2 via the exponent field) for 9 chunks,
    so neither engine paces the loop; effect^T accumulated as
    [v|1].T @ expT in bf16 (ones row gives the softmax denominator);
    PE-transpose back, normalize on ScalarE, add residual, DMA out.
  - PSUM: one 3-deep ring of [128,1024] fp32 tiles (6 banks) shared by
    logits, projection accumulators and the transpose scratch (bitcast
    view), + 2 banks for the effect accumulators.
  - The t-loop is software-pipelined (logits(t+1) issued before
    effect(t)) so the PE streams ahead of the exp engines.
"""

import math
import sys

for _p in ("/opt/trn_rl_repo", "/root/.axon_site/_ro/trn_rl_repo"):
    if _p not in sys.path:
        sys.path.append(_p)

import numpy as np

B, S, F = 4, 2048, 1024
H = 16
D = 64
G = 512            # feature columns per core (8 heads)
NH = 8             # heads per core
KC = 8             # contraction chunks of 128 over F
ST = 4             # s tiles of 512
TC = 16            # t chunks of 128
SCALE = 1.0 / math.sqrt(S / H)
# Schraudolph exp2-bit-trick constants for bf16 output: round(x*SM + SB) as
# int16 reinterpreted as bf16 approximates exp(SCALE*x). 2^7 scales into the
# bf16 exponent field; c=0.0579 centers the sawtooth error (~1.5% rms).
SCH_C = 0.0579
SCH_M = 128.0 * math.log2(math.e) * SCALE
SCH_B = 128.0 * (127.0 - SCH_C)
# VectorE takes fewer exp chunks than ScalarE because it also owns the
# per-head-pair epilogue (cast/reciprocal/normalize) and the proj copies.
# Its chunks sit late in the iteration so the deferred epilogue of the
# previous head pair (which runs at t=0..2) never delays an exp the PE is
# about to consume.
DVE_T = frozenset((5, 7, 9, 11, 13, 15))

_cached = None


def _build():
    import concourse.bacc as bacc
    import concourse.tile as tile
    from concourse import mybir
    from concourse.masks import make_identity

    dt = mybir.dt
    AF = mybir.ActivationFunctionType

    nc = bacc.Bacc("TRN2", target_bir_lowering=False, debug=False, num_devices=8)

    xt_d = nc.dram_tensor("xt", [128, KC, S], dt.bfloat16, kind="ExternalInput").ap()
    # Weights are host-packed [128, KC, G] (partition-major k-chunks) so each
    # matrix is ONE DMA: dma_start triggers cost ~650ns each on the issuing
    # engine, and 24 weight triggers were what delayed wv (and V-proj) by
    # ~14us at startup.
    wq_d = nc.dram_tensor("wq", [128, KC, G], dt.bfloat16, kind="ExternalInput").ap()
    wk_d = nc.dram_tensor("wk", [128, KC, G], dt.bfloat16, kind="ExternalInput").ap()
    wv_d = nc.dram_tensor("wv", [128, KC, G], dt.bfloat16, kind="ExternalInput").ap()
    res_d = nc.dram_tensor("res", [S, G], dt.bfloat16, kind="ExternalInput").ap()
    out_d = nc.dram_tensor("out", [S, G], dt.bfloat16, kind="ExternalOutput").ap()

    with tile.TileContext(nc) as tc:
        with tc.tile_pool(name="persist", bufs=1) as persist:
            identB = persist.tile([128, 128], dt.bfloat16)
            make_identity(nc, identB[:])
            ones = persist.tile([128, NH], dt.float32)
            nc.vector.memset(ones[:], 1.0)
            # Preload the exp table set on ScalarE while the DMAs run -- the
            # ~2.7us ACT_TABLE_LOAD otherwise lands mid-stream and idles PE
            # past the HAM window.
            scr = persist.tile([128, NH], dt.float32)
            nc.scalar.activation(scr[:], ones[:], AF.Exp)

            xTall = persist.tile([128, KC, S], dt.bfloat16, name="xT")
            xT = [xTall[:, k, :] for k in range(KC)]
            kT = [persist.tile([128, S], dt.bfloat16, name=f"kT{m}") for m in range(4)]
            vS = [persist.tile([128, NH, D + 1], dt.bfloat16, name=f"vS{t}")
                  for t in range(TC)]

            w_sb = {}
            with tc.tile_pool(name="wp", bufs=1) as wp:
                # DMA order is the consumption order of the prologue:
                #   sync:   xT(b0), wq, xT(b2), res...
                #   gpsimd: wk, xT(b1), xT(b3), wv
                # so K-proj(0,b) / Q-proj / V-proj each find their operands
                # just in time and the PE never drains waiting on X.
                def dma_w(nm, wd, eng):
                    t = wp.tile([128, KC, G], dt.bfloat16, name=f"w{nm}")
                    eng.dma_start(t[:], wd[:])
                    for k in range(KC):
                        w_sb[nm, k] = t[:, k]

                def dma_x(b, eng):
                    eng.dma_start(
                        xTall[:, :, b * 512:(b + 1) * 512],
                        xt_d[:, :, b * 512:(b + 1) * 512])

                dma_w("k", wk_d, nc.gpsimd)
                # b0 goes per-k-chunk so K-proj(0,0) streams behind the DMA
                # piece by piece (a single 1MB DMA completes ~14us in and
                # stalls the PE ~9us at startup); b1..b3 have lax deadlines
                # and stay single-trigger.
                for k in range(KC):
                    nc.sync.dma_start(xTall[:, k, 0:512], xt_d[:, k, 0:512])
                dma_x(1, nc.gpsimd)
                dma_w("q", wq_d, nc.sync)
                dma_w("v", wv_d, nc.gpsimd)
                dma_x(2, nc.sync)
                dma_x(3, nc.gpsimd)

                # PSUM: lp ring 3x2 banks (logits / proj accumulators / tp8
                # transpose scratch) + eps 2x1 banks = 8.
                with tc.tile_pool(name="pp", bufs=3, space="PSUM") as pp, \
                     tc.tile_pool(name="epp", bufs=2, space="PSUM") as epp, \
                     tc.tile_pool(name="qtsp", bufs=8) as qtsp, \
                     tc.tile_pool(name="expa", bufs=4) as expa, \
                     tc.tile_pool(name="expd", bufs=4) as expd, \
                     tc.tile_pool(name="esp", bufs=4) as esp, \
                     tc.tile_pool(name="stp", bufs=8) as stp, \
                     tc.tile_pool(name="rsp", bufs=3) as rsp, \
                     tc.tile_pool(name="rcp", bufs=4) as rcp:

                    def ptile():
                        return pp.tile([128, 1024], dt.float32, tag="lp", name="lp")

                    def emit_kproj(m, b):
                        pk = ptile()
                        for k in range(KC):
                            nc.tensor.matmul(
                                pk[:, 0:512], w_sb["k", k][:, m * 128:(m + 1) * 128],
                                xT[k][:, b * 512:(b + 1) * 512],
                                start=(k == 0), stop=(k == KC - 1))
                        nc.vector.tensor_copy(kT[m][:, b * 512:(b + 1) * 512],
                                              pk[:, 0:512])

                    def emit_vproj_j(b, j):
                        pv = ptile()
                        tci = b * 4 + j
                        for k in range(KC):
                            nc.tensor.matmul(
                                pv[:, 0:512],
                                xT[k][:, tci * 128:(tci + 1) * 128],
                                w_sb["v", k][:], start=(k == 0), stop=(k == KC - 1))
                        nc.vector.tensor_copy(
                            vS[tci][:, :, D:D + 1],
                            ones[:].rearrange("p (h o) -> p h o", o=1))
                        nc.vector.tensor_copy(
                            vS[tci][:, :, 0:D],
                            pv[:, 0:512].rearrange("p (h d) -> p h d", h=NH))

                    def emit_qproj(dst_st, m):
                        pq = ptile()
                        for k in range(KC):
                            nc.tensor.matmul(
                                pq[:, 0:512], w_sb["q", k][:, m * 128:(m + 1) * 128],
                                xT[k][:, dst_st * 512:(dst_st + 1) * 512],
                                start=(k == 0), stop=(k == KC - 1))
                        qt = qtsp.tile([128, 512], dt.bfloat16, tag="qts", name="qt")
                        nc.vector.tensor_copy(qt[:], pq[:, 0:512])
                        return qt

                    def emit_logits(hp, qts, t):
                        # one 2-bank psum tile holds both halves' logits for
                        # this t-chunk: the two K=64 matmuls run concurrently
                        # in disjoint PE row groups.
                        lp = ptile()
                        for half in range(2):
                            r0 = half * 64
                            nc.tensor.matmul(
                                lp[:, half * 512:(half + 1) * 512],
                                kT[hp][r0:r0 + 64, t * 128:(t + 1) * 128],
                                qts[r0:r0 + 64, :],
                                start=True, stop=True)
                        # exp: alternate engines so neither paces the loop.
                        if t in DVE_T:
                            exi = expd.tile([128, 1024], dt.int16, tag="exd", name="exi")
                            nc.vector.tensor_scalar(
                                exi[:], lp[:], SCH_M, SCH_B,
                                mybir.AluOpType.mult, mybir.AluOpType.add)
                            return exi[:].bitcast(dt.bfloat16)
                        ex = expa.tile([128, 1024], dt.bfloat16, tag="exa", name="ex")
                        nc.scalar.activation(ex[:], lp[:], AF.Exp, scale=SCALE)
                        return ex[:]

                    # ---- prologue, in DMA-arrival order: K-proj(0,b0) and
                    # Q-proj st=0 only need x-block b0 (+wk/wq, first on
                    # their queues); later K-proj blocks consume b1..b3 as
                    # they land; V-proj last (wv is the last weight DMA).
                    emit_kproj(0, 0)
                    qts_cur = [emit_qproj(0, m) for m in range(4)]
                    emit_kproj(0, 1)
                    emit_kproj(0, 2)
                    for j in range(4):
                        emit_vproj_j(0, j)
                    emit_kproj(0, 3)

                    # Remaining projections injected into the st=0 t-loops,
                    # one per t slot, each emitted before its first consumer:
                    # vproj(b,j) before effect(4b+j) of the NEXT head pair,
                    # kproj(m,*) anywhere before C(0,m) starts.
                    inject = {
                        (0, 0): dict(
                            [(4 * (b - 1) + j, [lambda b=b, j=j: emit_vproj_j(b, j)])
                             for b in range(1, 4) for j in range(4)] +
                            [(12 + b, [lambda b=b: emit_kproj(1, b)]) for b in range(4)]),
                        (0, 1): {2 + 4 * b: [lambda b=b: emit_kproj(2, b)]
                                 for b in range(4)},
                        (0, 2): {2 + 4 * b: [lambda b=b: emit_kproj(3, b)]
                                 for b in range(4)},
                    }

                    def make_epilogue(hp, ess, stage, fin=None, last=False):
                        # deferred epilogue tail (all VectorE, so cross-engine
                        # waits never sit at the head of ScalarE's exp FIFO):
                        # PE-transpose both halves into a psum-ring scratch
                        # (bitcast view), normalize by the ones-row
                        # denominator, write into stage. The effect^T bf16
                        # casts are NOT here -- they run at the owning
                        # iteration's end so the eps accumulator banks are
                        # free before the next iteration's effect(0). When
                        # this is hp=3's epilogue and the st-finish is
                        # pending, the residual-add + output DMA are
                        # interleaved j-major so the final DMAs start as
                        # early as possible.
                        def run():
                            tp8 = ptile()[:, 0:264].bitcast(dt.bfloat16) \
                                .rearrange("p (h c) -> p h c", c=D + 2)
                            for half in range(2):
                                for j in range(4):
                                    nc.tensor.transpose(
                                        tp8[:, half * 4 + j, 0:D + 1],
                                        ess[half][:, j * 128:(j + 1) * 128],
                                        identB[0:D + 1, 0:D + 1])
                            rec = rcp.tile([128, 8], dt.float32, tag="rec", name="rec")
                            nc.vector.reciprocal(rec[:], tp8[:, :, D])
                            for j in range(4):
                                for half in range(2):
                                    h = 2 * hp + half
                                    idx = half * 4 + j
                                    if last and half == 0:
                                        nc.scalar.activation(
                                            stage[j][:, h * 64:(h + 1) * 64],
                                            tp8[:, idx, 0:D], AF.Copy,
                                            scale=rec[:, idx:idx + 1])
                                    else:
                                        nc.vector.tensor_scalar_mul(
                                            stage[j][:, h * 64:(h + 1) * 64],
                                            tp8[:, idx, 0:D],
                                            rec[:, idx:idx + 1])
                                if fin is not None:
                                    s0, rts = fin
                                    nc.vector.tensor_add(stage[j][:], stage[j][:],
                                                         rts[j][:])
                                    nc.sync.dma_start(
                                        out_d[s0 + j * 128:s0 + (j + 1) * 128, :],
                                        stage[j][:])
                        return run

                    # The per-head-pair epilogue (and, for hp=3, the per-st
                    # residual+DMA) is deferred into the NEXT iteration's
                    # t-loop (t=0) so iteration boundaries never serialize
                    # the logits->exp->effect pipeline.
                    pend_epi = None
                    qts_next = [None] * 4
                    for st in range(ST):
                        s0 = st * 512
                        # Prefetch this tile's residual rows early (one DMA).
                        rtt = rsp.tile([128, 4, G], dt.bfloat16, tag="res", name="rt")
                        nc.sync.dma_start(
                            rtt[:], res_d[s0:s0 + 512, :]
                            .rearrange("(j p) g -> p j g", p=128))
                        rts = [rtt[:, j, :] for j in range(4)]
                        stage = [stp.tile([128, G], dt.bfloat16, tag="stage", name="stage")
                                 for _ in range(4)]
                        for hp in range(4):
                            inj = inject.get((st, hp), {})
                            eps = [epp.tile([D + 1, 512], dt.float32, tag="ep", name="ep")
                                   for _ in range(2)]
                            # software pipeline depth 2: logits(t+2) is
                            # emitted before effect(t) so the PE always has
                            # ~1.1us of queued work to cover the exp latency.
                            exq = [emit_logits(hp, qts_cur[hp], 0),
                                   emit_logits(hp, qts_cur[hp], 1)]
                            for t in range(TC):
                                for fn in inj.get(t, ()):
                                    fn()
                                if t < TC - 2:
                                    exq.append(emit_logits(hp, qts_cur[hp], t + 2))
                                if t == 0 and pend_epi is not None:
                                    pend_epi()
                                    pend_epi = None
                                if t == 8 and st < ST - 1:
                                    qts_next[hp] = emit_qproj(st + 1, hp)
                                ex_t = exq.pop(0)
                                for half in range(2):
                                    nc.tensor.matmul(
                                        eps[half][:],
                                        vS[t][:, 2 * hp + half, :],
                                        ex_t[:, half * 512:(half + 1) * 512],
                                        start=(t == 0), stop=(t == TC - 1))
                            # cast effect^T to bf16 NOW (frees the eps psum
                            # banks for the next iteration's effect(0)); the
                            # last iteration splits the casts across ScalarE
                            # and VectorE since nothing else is in flight.
                            last = st == ST - 1 and hp == 3
                            ess = []
                            for half in range(2):
                                es = esp.tile([D + 1, 512], dt.bfloat16,
                                              tag="es", name="es")
                                if last and half == 0:
                                    nc.scalar.copy(es[:], eps[half][:])
                                else:
                                    nc.vector.tensor_copy(es[:], eps[half][:])
                                ess.append(es)
                            pend_epi = make_epilogue(
                                hp, ess, stage,
                                fin=(s0, rts) if hp == 3 else None,
                                last=last)
                        qts_cur = qts_next
                        qts_next = [None] * 4
                    pend_epi()

    nc.compile()
    return nc


def _get_nc():
    global _cached
    if _cached is None:
        _cached = _build()
    return _cached


def _make_in_maps(X, residual_score, wq, wk, wv):
    import ml_dtypes

    bf16 = ml_dtypes.bfloat16

    def pack(a):
        # [F, cols] -> [128, KC, cols] partition-major k-chunks (one DMA).
        return np.ascontiguousarray(
            a.reshape(KC, 128, a.shape[1]).transpose(1, 0, 2))

    X = np.asarray(X, dtype=np.float32)
    residual_score = np.asarray(residual_score, dtype=np.float32).astype(bf16)
    wq = np.asarray(wq, dtype=np.float32).astype(bf16)
    wk = np.asarray(wk, dtype=np.float32).astype(bf16)
    wv = np.asarray(wv, dtype=np.float32).astype(bf16)
    xts = [pack(X[b].T.astype(bf16)) for b in range(B)]
    in_maps = []
    for c in range(8):
        b, g = c // 2, c % 2
        cols = slice(g * G, (g + 1) * G)
        in_maps.append({
            "xt": xts[b],
            "wq": pack(wq[:, cols]),
            "wk": pack(wk[:, cols]),
            "wv": pack(wv[:, cols]),
            "res": np.ascontiguousarray(residual_score[b, :, cols]),
        })
    return in_maps


def _assemble(results):
    out = np.empty((B, S, F), dtype=np.float32)
    for c in range(8):
        b, g = c // 2, c % 2
        out[b, :, g * G:(g + 1) * G] = np.asarray(results[c]["out"],
                                                  dtype=np.float32)
    return out


def run(X, residual_score, wq, wk, wv, trace=False):
    from concourse.bass_utils import run_bass_kernel_spmd

    nc = _get_nc()
    in_maps = _make_in_maps(X, residual_score, wq, wk, wv)
    res = run_bass_kernel_spmd(nc, in_maps, core_ids=list(range(8)), trace=trace)
    return _assemble(res.results), res


def kernel(X, residual_score, wq, wk, wv):
    out, _ = run(X, residual_score, wq, wk, wv)
    return (out, out)
